# revision 1
# baseline (speedup 1.0000x reference)
"""Trainium2 Bass kernel for BiologicalMultiHeadAttention.

Sharding (8 cores): core c -> (batch b = c//2, head-group g = c%2).
Each core computes, for its batch and its 8 heads (512 channels):
  q/k/v projections, dense softmax attention, neuromodulation gate,
  and a partial output projection over its 512 channels.
Host sums the two partial projections per batch and adds bo.

On-chip layout is "transposed activations" [channels, seq] so every
matmul has K on partitions; the host pre-transposes x and the weights
(numpy) and casts to bf16.

Emission order is tuned so the ACT-bound attention phase hides the
projection / MLP / out-projection matmuls in PE slack: q/k chunk p is
emitted just before the head pair that needs it, v streams inside head
0's loop, and each half's normalize + out-projection hides under the
other half's attention.
"""

import os
import numpy as np
import ml_dtypes

import concourse.bass as bass
import concourse.tile as tile
from concourse import bacc, mybir
from concourse.bass_utils import run_bass_kernel_spmd

F32 = mybir.dt.float32
BF16 = mybir.dt.bfloat16
AF = mybir.ActivationFunctionType
ALU = mybir.AluOpType

P = 128


def build_nc(S=2048, E=1024, HL=8, D=64, num_devices=8):
    """Per-core program. HL = heads per core."""
    CH = HL * D            # output channels per core (512)
    NE = E // P            # xT channel chunks (8)
    NC = CH // P           # qT/kT channel chunks (4)
    NS = S // P            # seq chunks (16)
    HM = E // 4            # mlp hidden (256)
    NH = HM // P           # h1T chunks (2)
    HALF = min(S, 1024)    # sq span per attention inner block
    NHALF = S // HALF
    TT = min(HALF, 512)    # matmul free-dim tile
    NT = HALF // TT        # n-tiles per half
    HPC = P // D           # heads per channel chunk (2)
    stW = HALF // P

    nc = bacc.Bacc("TRN2", target_bir_lowering=False, debug=False,
                   num_devices=num_devices)

    xT_d = nc.dram_tensor("xT", [E, S], BF16, kind="ExternalInput").ap()
    wqT_d = nc.dram_tensor("wqT", [E, CH], BF16, kind="ExternalInput").ap()
    wkT_d = nc.dram_tensor("wkT", [E, CH], BF16, kind="ExternalInput").ap()
    wvT_d = nc.dram_tensor("wvT", [E, CH], BF16, kind="ExternalInput").ap()
    wm1T_d = nc.dram_tensor("wm1T", [E, HM], BF16, kind="ExternalInput").ap()
    wm2T_d = nc.dram_tensor("wm2T", [HM, CH], BF16, kind="ExternalInput").ap()
    wo_d = nc.dram_tensor("wo", [CH, E], BF16, kind="ExternalInput").ap()
    bq_d = nc.dram_tensor("bq", [CH], F32, kind="ExternalInput").ap()
    bk_d = nc.dram_tensor("bk", [CH], F32, kind="ExternalInput").ap()
    bvr_d = nc.dram_tensor("bvr", [P, CH], F32, kind="ExternalInput").ap()
    bm1_d = nc.dram_tensor("bm1", [HM], F32, kind="ExternalInput").ap()
    bm2_d = nc.dram_tensor("bm2", [CH], F32, kind="ExternalInput").ap()
    # scal columns: dopamine, serotonin, norepinephrine, acetylcholine,
    # attn_scale, attn_bias, 0, 0 (replicated over 128 partitions by host)
    scal_d = nc.dram_tensor("scal", [P, 8], F32, kind="ExternalInput").ap()
    sel_d = nc.dram_tensor("sel", [P // D, P], F32, kind="ExternalInput").ap()
    out_d = nc.dram_tensor("out", [S, E], F32, kind="ExternalOutput").ap()

    with tile.TileContext(nc) as tc:
        with (
            tc.tile_pool(name="const", bufs=1) as const,
            tc.tile_pool(name="xp", bufs=1) as xp,
            tc.tile_pool(name="expp", bufs=4) as expp,
            tc.tile_pool(name="evp", bufs=3) as evp,
            tc.tile_pool(name="denp", bufs=2) as denp,
            tc.tile_pool(name="rdp", bufs=2) as rdp,
            tc.tile_pool(name="tailp", bufs=2) as tailp,
            tc.tile_pool(name="outp", bufs=2) as outp,
            tc.tile_pool(name="ps", bufs=3, space="PSUM") as ps,
            tc.tile_pool(name="acc", bufs=1, space="PSUM") as accp,
        ):
            # ---------------- static tiles + loads ----------------
            xT = xp.tile([P, NE, S], BF16)
            for o in range(NE):
                nc.sync.dma_start(
                    xT[:, o, :],
                    xT_d.rearrange("(o p) f -> o p f", p=P)[o])

            def load_w(pool, dram, chunks, width, name):
                t = pool.tile([P, chunks, width], BF16, tag=name)
                nc.sync.dma_start(
                    t[:], dram.rearrange("(o p) f -> p o f", p=P))
                return t

            wqT = load_w(xp, wqT_d, NE, CH, "wqT")
            wkT = load_w(xp, wkT_d, NE, CH, "wkT")
            wvT = load_w(xp, wvT_d, NE, CH, "wvT")
            wm1T = load_w(xp, wm1T_d, NE, HM, "wm1T")
            wm2T = load_w(xp, wm2T_d, NH, CH, "wm2T")
            wo = load_w(const, wo_d, NC, E, "wo")

            def load_b(dram, chunks, name):
                t = const.tile([P, chunks], F32, tag=name)
                nc.sync.dma_start(t[:], dram.rearrange("(c p) -> p c", p=P))
                return t

            bq = load_b(bq_d, NC, "bq")
            bk = load_b(bk_d, NC, "bk")
            bm1 = load_b(bm1_d, NH, "bm1")
            bm2 = load_b(bm2_d, NC, "bm2")

            bv_bc = const.tile([P, CH], F32, tag="bv_bc")
            nc.sync.dma_start(bv_bc[:], bvr_d)

            # selector for pair-broadcast of rdenom rows: out = sel.T @ rows
            sel = const.tile([HPC, P], F32, tag="sel")
            nc.sync.dma_start(sel[:], sel_d)

            scal = const.tile([P, 8], F32, tag="scal")
            nc.sync.dma_start(scal[:], scal_d)

            # nm_gain = (dop + ser + nor + ace) / 4  -> [128, 1]
            nm = const.tile([P, 2], F32, tag="nm")
            nc.vector.tensor_tensor(nm[:, 0:1], scal[:, 0:1], scal[:, 1:2], ALU.add)
            nc.vector.tensor_tensor(nm[:, 1:2], scal[:, 2:3], scal[:, 3:4], ALU.add)
            nc.vector.tensor_tensor(nm[:, 0:1], nm[:, 0:1], nm[:, 1:2], ALU.add)
            nc.vector.tensor_scalar_mul(nm[:, 0:1], nm[:, 0:1], 0.25)
            nm_g = nm[:, 0:1]
            a_scale = scal[:, 4:5]
            a_bias = scal[:, 5:6]

            # c1[m] = 1 + nm * bm2[m]  (per channel chunk) -> gate affine
            c1 = const.tile([P, NC], F32, tag="c1")
            nc.vector.tensor_tensor(c1[:], bm2[:], nm_g.to_broadcast([P, NC]), ALU.mult)
            nc.vector.tensor_scalar_add(c1[:], c1[:], 1.0)

            # ---------------- persistent activations --------------------
            qT = const.tile([P, NC, S], BF16, tag="qT")
            kT = const.tile([P, NC, S], BF16, tag="kT")
            v_aug = const.tile([P, NS, HL, D + 1], BF16, tag="v_aug")
            h1T = const.tile([P, NH, S], BF16, tag="h1T")
            gateT = const.tile([P, NC, S], BF16, tag="gateT")
            # attn_raw doubles as finalT: normalization rewrites it in place
            attn_raw = const.tile([P, NC, S], BF16, tag="attn_raw")
            rstage = const.tile([P, HL, NHALF, stW], F32, tag="rstage")

            nc.vector.memset(v_aug[:, :, :, D:D + 1], 1.0)

            SQB = min(1024, S)     # seq block per psum-evict group
            NSQB = S // SQB
            NBT = SQB // 512 if SQB >= 512 else 1
            BT = min(512, SQB)     # matmul free tile within block

            def proj_chunk(wT, dest, bias, m, kchunks, src,
                           relu=False, gate=False):
                # dest[ch chunk m, :] = (wT.T @ src) + bias
                for t2 in range(NSQB):
                    pt = ps.tile([P, SQB], F32, tag="ps", name=f"pj_{m}_{t2}")
                    for n in range(NBT):
                        sl = slice(n * BT, n * BT + BT)
                        col = slice(t2 * SQB + n * BT, t2 * SQB + (n + 1) * BT)
                        for k in range(kchunks):
                            nc.tensor.matmul(
                                pt[:, sl],
                                wT[:, k, m * P:(m + 1) * P],
                                src[:, k, col],
                                start=(k == 0), stop=(k == kchunks - 1))
                    dsl = slice(t2 * SQB, (t2 + 1) * SQB)
                    if relu:
                        nc.scalar.activation(
                            dest[:, m, dsl], pt[:], AF.Relu,
                            bias=bias[:, m:m + 1])
                    elif gate:
                        nc.vector.tensor_scalar(
                            dest[:, m, dsl], pt[:], nm_g, c1[:, m:m + 1],
                            ALU.mult, ALU.add)
                    else:
                        nc.vector.tensor_scalar_add(
                            dest[:, m, dsl], pt[:], bias[:, m:m + 1])

            def emit_qk(m):
                proj_chunk(wqT, qT, bq, m, NE, xT)
                proj_chunk(wkT, kT, bk, m, NE, xT)

            def emit_v_chunk(c):
                # v natural layout [seq, ch] + ones column
                pt = ps.tile([P, SQB], F32, tag="ps", name=f"v_{c}")
                for k in range(NE):
                    nc.tensor.matmul(
                        pt[:, 0:CH], xT[:, k, c * P:(c + 1) * P], wvT[:, k, :],
                        start=(k == 0), stop=(k == NE - 1))
                nc.vector.tensor_tensor(
                    v_aug[:, c, :, 0:D],
                    pt[:, 0:CH].rearrange("p (h d) -> p h d", h=HL),
                    bv_bc.rearrange("p (h d) -> p h d", h=HL),
                    ALU.add)

            scale = float(D) ** -0.5

            # work queue: one ~512-col psum group emitted per attention
            # iteration, so projection matmuls rotate fairly through the
            # shared psum slots and hide in ACT-bound slack
            from collections import deque
            pending = deque()

            def queue_proj_chunk(wT, dest, bias, m, kchunks, src_t,
                                 relu=False, gate=False):
                for t4 in range(S // 512):
                    def emit(t4=t4, wT=wT, dest=dest, bias=bias, m=m,
                             kchunks=kchunks, src_t=src_t, relu=relu,
                             gate=gate):
                        pt = ps.tile([P, 512], F32, tag="ps",
                                     name=f"pj_{dest.tensor.name}_{m}_{t4}")
                        col = slice(t4 * 512, (t4 + 1) * 512)
                        for k in range(kchunks):
                            nc.tensor.matmul(
                                pt[:], wT[:, k, m * P:(m + 1) * P],
                                src_t[:, k, col],
                                start=(k == 0), stop=(k == kchunks - 1))
                        if relu:
                            nc.scalar.activation(
                                dest[:, m, col], pt[:], AF.Relu,
                                bias=bias[:, m:m + 1])
                        elif gate:
                            nc.vector.tensor_scalar(
                                dest[:, m, col], pt[:], nm_g, c1[:, m:m + 1],
                                ALU.mult, ALU.add)
                        else:
                            nc.vector.tensor_scalar_add(
                                dest[:, m, col], pt[:], bias[:, m:m + 1])
                    pending.append(emit)

            def queue_qk(m):
                proj_chunk(wqT, qT, bq, m, NE, xT)
                proj_chunk(wkT, kT, bk, m, NE, xT)

            def emit_v_chunk(c):
                # v natural layout [seq, ch] + ones column
                pt = ps.tile([P, 512], F32, tag="ps", name=f"v_{c}")
                for k in range(NE):
                    nc.tensor.matmul(
                        pt[:, 0:CH], xT[:, k, c * P:(c + 1) * P], wvT[:, k, :],
                        start=(k == 0), stop=(k == NE - 1))
                nc.vector.tensor_tensor(
                    v_aug[:, c, :, 0:D],
                    pt[:, 0:CH].rearrange("p (h d) -> p h d", h=HL),
                    bv_bc.rearrange("p (h d) -> p h d", h=HL),
                    ALU.add)

            def queue_outproj(half):
                for t in range(half * NS // NHALF, (half + 1) * NS // NHALF):
                    ot = outp.tile([P, E], F32, tag="osb", name=f"osb_{t}")
                    for n in range(E // 512):
                        pt = ps.tile([P, SQB], F32, tag="ps", name=f"op_{t}_{n}")
                        for k in range(NC):
                            nc.tensor.matmul(
                                pt[:, 0:512],
                                attn_raw[:, k, t * P:(t + 1) * P],
                                wo[:, k, n * 512:(n + 1) * 512],
                                start=(k == 0), stop=(k == NC - 1))
                        nc.vector.tensor_copy(ot[:, n * 512:(n + 1) * 512],
                                              pt[:, 0:512])
                    nc.sync.dma_start(out_d[t * P:(t + 1) * P, :], ot[:])

            def head_evict(h, half, acc):
                # raw attention (bf16) + denominator row (f32) to SBUF;
                # SBUF->SBUF DMAs place odd heads at partitions 64..127
                ch = h // HPC
                kqp = (h % HPC) * D
                tmp = evp.tile([D, HALF], BF16, tag="ev", name=f"ev_{h}_{half}")
                nc.vector.tensor_copy(tmp[:], acc[0:D, :])
                den = denp.tile([1, HALF], F32, tag="den", name=f"dn_{h}_{half}")
                nc.vector.tensor_copy(den[:], acc[D:D + 1, :])
                nc.sync.dma_start(
                    attn_raw[kqp:kqp + D, ch, half * HALF:(half + 1) * HALF],
                    tmp[:])
                nc.sync.dma_start(rstage[:, h, half, :], den[:])

            def attn_unit(h, half, pre_j=None):
                ch = h // HPC
                kqp = (h % HPC) * D
                acc = accp.tile([D + 1, HALF], F32, tag="acc",
                                name=f"acc_{h}_{half}")
                for j in range(NS):
                    sc = ps.tile([P, HALF], F32, tag="ps",
                                 name=f"sc_{h}_{half}_{j}")
                    lhs_k = kT[kqp:kqp + D, ch, j * P:(j + 1) * P]
                    for n in range(NT):
                        nc.tensor.matmul(
                            sc[:, n * TT:(n + 1) * TT],
                            lhs_k,
                            qT[kqp:kqp + D, ch,
                               half * HALF + n * TT:half * HALF + (n + 1) * TT],
                            start=True, stop=True)
                    ex = expp.tile([P, HALF], BF16, tag="ex",
                                   name=f"ex_{h}_{half}_{j}")
                    nc.scalar.activation(ex[:], sc[:], AF.Exp, scale=scale)
                    if pre_j is not None:
                        pre_j(j)
                    elif pending:
                        pending.popleft()()
                    for n in range(NT):
                        nc.tensor.matmul(
                            acc[:, n * TT:(n + 1) * TT],
                            v_aug[:, j, h, :],
                            ex[:, n * TT:(n + 1) * TT],
                            start=(j == 0), stop=(j == NS - 1))
                head_evict(h, half, acc)

            def tail_norm(half):
                hsl = slice(half * HALF, (half + 1) * HALF)
                nc.vector.reciprocal(rstage[:, :, half, :], rstage[:, :, half, :])
                nc.vector.tensor_scalar_mul(
                    rstage[:, :, half, :], rstage[:, :, half, :], a_scale)
                for pr in range(NC):          # head pair == channel chunk
                    rd = rdp.tile([HPC, HALF], F32, tag="rd",
                                  name=f"rd_{pr}_{half}")
                    for hp in range(HPC):
                        nc.sync.dma_start(
                            rd[hp:hp + 1, :],
                            rstage[:, pr * HPC + hp, half, :])
                    bc = accp.tile([P, HALF], F32, tag="acc",
                                   name=f"bc_{pr}_{half}")
                    for n in range(NT):
                        nc.tensor.matmul(
                            bc[:, n * TT:(n + 1) * TT], sel[:],
                            rd[:, n * TT:(n + 1) * TT],
                            start=True, stop=True)
                    t1 = tailp.tile([P, HALF], F32, tag="t1",
                                    name=f"t1_{pr}_{half}")
                    nc.vector.tensor_tensor(
                        t1[:], attn_raw[:, pr, hsl], bc[:], ALU.mult)
                    nc.vector.tensor_scalar_add(t1[:], t1[:], a_bias)
                    nc.vector.tensor_tensor(
                        attn_raw[:, pr, hsl], t1[:], gateT[:, pr, hsl], ALU.mult)

            # ---------------- emission schedule ----------------
            assert HL == 8 and NC == 4 and NHALF == 2

            def drain():
                while pending:
                    pending.popleft()()

            # q/k chunk 0 up front (heads 0-1 need only it); the rest
            # of the projections drip one 512-group per attention iteration
            # through the third psum slot
            queue_qk(0)
            attn_unit(0, 0, pre_j=lambda j: emit_v_chunk(j))
            queue_proj_chunk(wqT, qT, bq, 1, NE, xT)
            queue_proj_chunk(wkT, kT, bk, 1, NE, xT)
            attn_unit(1, 0)
            queue_proj_chunk(wqT, qT, bq, 2, NE, xT)
            queue_proj_chunk(wkT, kT, bk, 2, NE, xT)
            attn_unit(2, 0)
            queue_proj_chunk(wqT, qT, bq, 3, NE, xT)
            queue_proj_chunk(wkT, kT, bk, 3, NE, xT)
            attn_unit(3, 0)
            for m in range(NH):
                queue_proj_chunk(wm1T, h1T, bm1, m, NE, xT, relu=True)
            attn_unit(4, 0)
            for m in range(NC):
                queue_proj_chunk(wm2T, gateT, bm2, m, NH, h1T, gate=True)
            attn_unit(5, 0)
            attn_unit(6, 0)
            attn_unit(7, 0)
            drain()
            tail_norm(0)
            for h in range(HL):
                attn_unit(h, 1)
            queue_outproj(0)
            drain()
            tail_norm(1)
            queue_outproj(1)
            drain()

    nc.compile()
    return nc


_CACHE = {}


def _get_nc():
    if "nc" not in _CACHE:
        _CACHE["nc"] = build_nc()
    return _CACHE["nc"]


def _bf16_t(a):
    """transpose + cast to contiguous bf16"""
    return np.ascontiguousarray(np.asarray(a, np.float32).T).astype(ml_dtypes.bfloat16)


def kernel(query, Wq, bq, Wk, bk, Wv, bv, Wo, bo,
           Wm1, bm1, Wm2, bm2,
           dopamine, serotonin, norepinephrine, acetylcholine,
           attn_scale, attn_bias):
    B, S, E = 4, 2048, 1024
    CH = 512
    nc = _get_nc()

    query = np.asarray(query, np.float32)
    f32 = lambda a: np.ascontiguousarray(np.asarray(a, np.float32))
    scal_row = np.array([float(np.asarray(dopamine).reshape(-1)[0]),
                         float(np.asarray(serotonin).reshape(-1)[0]),
                         float(np.asarray(norepinephrine).reshape(-1)[0]),
                         float(np.asarray(acetylcholine).reshape(-1)[0]),
                         float(np.asarray(attn_scale).reshape(-1)[0]),
                         float(np.asarray(attn_bias).reshape(-1)[0]),
                         0.0, 0.0], np.float32)
    scal = np.tile(scal_row[None, :], (128, 1))
    D_ = 64
    sel = np.zeros((128 // D_, 128), np.float32)
    sel[0, 0:D_] = 1.0
    sel[1, D_:2 * D_] = 1.0

    wm1T = _bf16_t(Wm1)
    in_maps = []
    for core in range(8):
        b, g = core // 2, core % 2
        cg = slice(g * CH, (g + 1) * CH)
        Wo_np = np.asarray(Wo, np.float32)
        in_maps.append({
            "xT": _bf16_t(query[b]),
            "wqT": _bf16_t(np.asarray(Wq, np.float32)[cg]),
            "wkT": _bf16_t(np.asarray(Wk, np.float32)[cg]),
            "wvT": _bf16_t(np.asarray(Wv, np.float32)[cg]),
            "wm1T": wm1T,
            "wm2T": _bf16_t(np.asarray(Wm2, np.float32)[cg]),
            "wo": _bf16_t(Wo_np[:, cg]),
            "bq": f32(np.asarray(bq, np.float32)[cg]),
            "bk": f32(np.asarray(bk, np.float32)[cg]),
            "bvr": np.ascontiguousarray(
                np.tile(np.asarray(bv, np.float32)[cg][None, :], (128, 1))),
            "bm1": f32(bm1),
            "bm2": f32(np.asarray(bm2, np.float32)[cg]),
            "scal": scal,
            "sel": sel,
        })

    res = run_bass_kernel_spmd(nc, in_maps, core_ids=list(range(8)))
    _CACHE["last_results"] = res

    bo_np = np.asarray(bo, np.float32)
    out = np.empty((B, S, E), np.float32)
    for b in range(B):
        out[b] = res.results[2 * b]["out"] + res.results[2 * b + 1]["out"] + bo_np
    return out



# revision 10
# speedup vs baseline: 1.1582x; 1.1582x over previous
"""Trainium2 Bass kernel for BiologicalMultiHeadAttention.

Sharding (8 cores): core c -> (batch b = c//2, head-group g = c%2).
Each core: q/k/v projections, dense softmax attention over its 8 heads,
neuromodulation gate, partial out-projection over its 512 channels.
Host sums the two partial projections per batch and adds bo.

Design: the softmax exp stream on ScalarE (~283us for 33.5M exps) is the
critical path; PE work is scheduled to hide under it.
  - scores: bf16, both heads of a pair packed concurrently in the PE
    array (row-group tiling at partitions 0/64, K=64 each) -> ~2x.
  - all data bf16 (fp8 fails the 2e-2 budget: each quantized tensor in
    the multiplicative path contributes its full ~3.6% elementwise RMS).
  - loop: head-pair outer, 512-query blocks; per key-chunk jc:
    2 packed score MMs -> one exp (N=1024, psum ping-pong) -> attn*v.
    Ones-column in v gives the denominator row for free (M=65).
  - projections / gate / normalize / out-projection drip through an
    ordered pending queue, one item per jc step.
"""

import numpy as np
import ml_dtypes
from collections import deque

import concourse.bass as bass
import concourse.tile as tile
from concourse import bacc, mybir
from concourse.bass_utils import run_bass_kernel_spmd

F32 = mybir.dt.float32
F32R = mybir.dt.float32r
BF16 = mybir.dt.bfloat16
AF = mybir.ActivationFunctionType
ALU = mybir.AluOpType

P = 128


def build_nc(S=2048, E=1024, HL=8, D=64, num_devices=8):
    CH = HL * D        # 512 channels per core
    NE = E // P        # 8 input-channel chunks
    NC = CH // P       # 4 output chunks (= head pairs)
    NS = S // P        # 16 key chunks
    HM = E // 4        # 256 mlp hidden
    NH = HM // P       # 2
    QB = 512           # query block
    NQB = S // QB      # 4

    nc = bacc.Bacc("TRN2", target_bir_lowering=False, debug=False,
                   num_devices=num_devices)

    xT_d = nc.dram_tensor("xT", [E, S], BF16, kind="ExternalInput").ap()
    wqT_d = nc.dram_tensor("wqT", [E, CH], BF16, kind="ExternalInput").ap()
    wkT_d = nc.dram_tensor("wkT", [E, CH], BF16, kind="ExternalInput").ap()
    wvT_d = nc.dram_tensor("wvT", [E, CH], BF16, kind="ExternalInput").ap()
    wm1T_d = nc.dram_tensor("wm1T", [E, HM], BF16, kind="ExternalInput").ap()
    wm2T_d = nc.dram_tensor("wm2T", [HM, CH], BF16, kind="ExternalInput").ap()
    wo_d = nc.dram_tensor("wo", [CH, E], BF16, kind="ExternalInput").ap()
    bq_d = nc.dram_tensor("bq", [CH], F32, kind="ExternalInput").ap()
    bk_d = nc.dram_tensor("bk", [CH], F32, kind="ExternalInput").ap()
    bvr_d = nc.dram_tensor("bvr", [P, CH], F32, kind="ExternalInput").ap()
    bm1_d = nc.dram_tensor("bm1", [HM], F32, kind="ExternalInput").ap()
    bm2_d = nc.dram_tensor("bm2", [CH], F32, kind="ExternalInput").ap()
    # scal cols: dop, ser, nor, ace, attn_scale, attn_bias, 0, 0
    scal_d = nc.dram_tensor("scal", [P, 8], F32, kind="ExternalInput").ap()
    sel_d = nc.dram_tensor("sel", [2, P], F32, kind="ExternalInput").ap()
    out_d = nc.dram_tensor("out", [S, E], F32, kind="ExternalOutput").ap()

    with tile.TileContext(nc) as tc:
        with (
            tc.tile_pool(name="const", bufs=1) as const,
            tc.tile_pool(name="exp", bufs=3) as exp_pool,
            tc.tile_pool(name="evp", bufs=2) as evp,
            tc.tile_pool(name="denp", bufs=2) as denp,
            tc.tile_pool(name="rdp", bufs=2) as rdp,
            tc.tile_pool(name="t1p", bufs=2) as t1p,
            tc.tile_pool(name="osp", bufs=2) as osp,
            tc.tile_pool(name="scp", bufs=2, space="PSUM") as scp,
            tc.tile_pool(name="accp", bufs=2, space="PSUM") as accp,
            tc.tile_pool(name="ps", bufs=2, space="PSUM") as ps,
        ):
            # ---------------- loads ----------------
            xT = const.tile([P, NE, S], BF16, tag="xT")
            for o in range(NE):
                nc.sync.dma_start(
                    xT[:, o, :],
                    xT_d.rearrange("(o p) f -> o p f", p=P)[o])

            def load_w(dram, chunks, width, name):
                t = const.tile([P, chunks, width], BF16, tag=name)
                nc.sync.dma_start(
                    t[:], dram.rearrange("(o p) f -> p o f", p=P))
                return t

            wqT = load_w(wqT_d, NE, CH, "wqT")
            wkT = load_w(wkT_d, NE, CH, "wkT")
            wvT = load_w(wvT_d, NE, CH, "wvT")
            wm1T = load_w(wm1T_d, NE, HM, "wm1T")
            wm2T = load_w(wm2T_d, NH, CH, "wm2T")
            wo = load_w(wo_d, NC, E, "wo")

            def load_b(dram, chunks, name):
                t = const.tile([P, chunks], F32, tag=name)
                nc.sync.dma_start(t[:], dram.rearrange("(c p) -> p c", p=P))
                return t

            bq = load_b(bq_d, NC, "bq")
            bk = load_b(bk_d, NC, "bk")
            bm1 = load_b(bm1_d, NH, "bm1")
            bm2 = load_b(bm2_d, NC, "bm2")

            bv_bc = const.tile([P, CH], F32, tag="bv_bc")
            nc.sync.dma_start(bv_bc[:], bvr_d)

            sel_raw = const.tile([2, P], F32, tag="sel_raw")
            nc.sync.dma_start(sel_raw[:], sel_d)

            scal = const.tile([P, 8], F32, tag="scal")
            nc.sync.dma_start(scal[:], scal_d)

            # ---------------- scalar-derived constants ----------------
            # nm = (dop+ser+nor+ace)/4
            nm = const.tile([P, 2], F32, tag="nm")
            nc.vector.tensor_tensor(nm[:, 0:1], scal[:, 0:1], scal[:, 1:2], ALU.add)
            nc.vector.tensor_tensor(nm[:, 1:2], scal[:, 2:3], scal[:, 3:4], ALU.add)
            nc.vector.tensor_tensor(nm[:, 0:1], nm[:, 0:1], nm[:, 1:2], ALU.add)
            nc.vector.tensor_scalar_mul(nm[:, 0:1], nm[:, 0:1], 0.25)
            nm_g = nm[:, 0:1]

            # gate = psum*nm + (1 + nm*bm2)
            c1 = const.tile([P, NC], F32, tag="c1")
            nc.vector.tensor_tensor(c1[:], bm2[:], nm_g.to_broadcast([P, NC]), ALU.mult)
            nc.vector.tensor_scalar_add(c1[:], c1[:], 1.0)

            # bc = sel.T @ rd(=1/den rows) scaled by attn_scale
            sel_s = const.tile([2, P], F32R, tag="sel_s")
            nc.vector.tensor_scalar(sel_s[:], sel_raw[:], scal[0:2, 4:5],
                                    None, ALU.mult)
            ab1 = const.tile([P, 1], F32, tag="ab1")
            nc.vector.tensor_copy(ab1[:], scal[:, 5:6])

            # ---------------- persistent activations ----------------
            qT = const.tile([P, NC, S], BF16, tag="qT")
            kT = const.tile([P, NC, S], BF16, tag="kT")
            v_aug = const.tile([P, NS, HL, D + 1], BF16, tag="v_aug")
            h1T = const.tile([P, NH, S], BF16, tag="h1T")
            gateT = const.tile([P, NC, S], BF16, tag="gateT")
            attn_raw = const.tile([P, NC, S], BF16, tag="attn_raw")
            attn_n = const.tile([P, NC, S], BF16, tag="attn_n")
            rstage = const.tile([P, HL, NQB, QB // P], F32R, tag="rstage")

            nc.vector.memset(v_aug[:, :, :, D:D + 1], 1.0)

            # ---------------- pending work queue ----------------
            pending = deque()   # (label, fn)
            emitted = set()

            def push(label, fn):
                pending.append((label, fn))

            def drip(n=1):
                for _ in range(n):
                    if pending:
                        label, fn = pending.popleft()
                        fn()
                        emitted.add(label)

            def ensure(label):
                while pending and label not in emitted:
                    lb, fn = pending.popleft()
                    fn()
                    emitted.add(lb)

            def drain():
                while pending:
                    lb, fn = pending.popleft()
                    fn()
                    emitted.add(lb)

            # ---------------- emitters ----------------
            # proj groups are split in two halves (4 k-chunks each) so a
            # drip item is ~0.9us of PE work.
            def kq_half(wT, m, g, half, cell, name):
                cols = slice(g * 512, (g + 1) * 512)
                if half == 0:
                    cell["pt"] = ps.tile([P, 512], F32, tag="ps",
                                         name=f"pj_{name}_{m}_{g}")
                pt = cell["pt"]
                for k in range(half * 4, half * 4 + 4):
                    nc.tensor.matmul(
                        pt[:], wT[:, k, m * P:(m + 1) * P],
                        xT[:, k, cols],
                        start=(k == 0), stop=(k == NE - 1))

            def kq_evict(dest, bias, m, g, cell):
                cols = slice(g * 512, (g + 1) * 512)
                nc.vector.tensor_scalar(
                    dest[:, m, cols], cell.pop("pt")[:], bias[:, m:m + 1],
                    None, ALU.add)

            def emit_kq_group(wT, dest, bias, m, g, name):
                cell = {}
                kq_half(wT, m, g, 0, cell, name)
                kq_half(wT, m, g, 1, cell, name)
                kq_evict(dest, bias, m, g, cell)

            def push_kq_group(wT, dest, bias, m, g, name):
                cell = {}
                push(f"{name}{m}g{g}a",
                     lambda: kq_half(wT, m, g, 0, cell, name))
                push(f"{name}{m}g{g}",
                     lambda: (kq_half(wT, m, g, 1, cell, name),
                              kq_evict(dest, bias, m, g, cell)))

            def emit_v_chunk(c):
                # v natural layout [seq, ch] + bias
                pt = ps.tile([P, 512], F32, tag="ps", name=f"v_{c}")
                for k in range(NE):
                    nc.tensor.matmul(
                        pt[:, 0:CH], xT[:, k, c * P:(c + 1) * P],
                        wvT[:, k, :],
                        start=(k == 0), stop=(k == NE - 1))
                nc.vector.tensor_tensor(
                    v_aug[:, c, :, 0:D],
                    pt[:, 0:CH].rearrange("p (h d) -> p h d", h=HL),
                    bv_bc.rearrange("p (h d) -> p h d", h=HL),
                    ALU.add)

            def h1_group(m, g, half, cell):
                cols = slice(g * 512, (g + 1) * 512)
                if half == 0:
                    cell["pt"] = ps.tile([P, 512], F32, tag="ps",
                                         name=f"h1_{m}_{g}")
                pt = cell["pt"]
                for k in range(half * 4, half * 4 + 4):
                    nc.tensor.matmul(
                        pt[:], wm1T[:, k, m * P:(m + 1) * P],
                        xT[:, k, cols],
                        start=(k == 0), stop=(k == NE - 1))
                if half == 1:
                    nc.vector.tensor_scalar(
                        h1T[:, m, cols], cell.pop("pt")[:],
                        bm1[:, m:m + 1], 0.0, ALU.add, ALU.max)

            def push_h1(g):
                for m in range(NH):
                    cell = {}
                    push(f"h1m{m}g{g}a",
                         lambda m=m, g=g, cell=cell: h1_group(m, g, 0, cell))
                    push(f"h1m{m}g{g}",
                         lambda m=m, g=g, cell=cell: h1_group(m, g, 1, cell))

            def emit_gate_group(m, g):
                cols = slice(g * 512, (g + 1) * 512)
                pt = ps.tile([P, 512], F32, tag="ps", name=f"g_{m}_{g}")
                for k in range(NH):
                    nc.tensor.matmul(
                        pt[:], wm2T[:, k, m * P:(m + 1) * P],
                        h1T[:, k, cols],
                        start=(k == 0), stop=(k == NH - 1))
                nc.vector.tensor_scalar(
                    gateT[:, m, cols], pt[:], nm_g, c1[:, m:m + 1],
                    ALU.mult, ALU.add)

            def push_gate(m, g):
                push(f"gm{m}g{g}", lambda m=m, g=g: emit_gate_group(m, g))

            def emit_tail1(pr, qb):
                sl2 = slice(2 * pr, 2 * pr + 2)
                with nc.allow_low_precision(reason="f32r==f32 bits; PE reads fp22"):
                    nc.vector.reciprocal(rstage[:, sl2, qb, :],
                                         rstage[:, sl2, qb, :])
                rd = rdp.tile([2, QB], F32R, tag="rd", name=f"rd_{pr}_{qb}")
                for hp in range(2):
                    nc.sync.dma_start(rd[hp:hp + 1, :],
                                      rstage[:, 2 * pr + hp, qb, :])
                return rd

            def emit_tail2(pr, qb, rd):
                qsl = slice(qb * QB, (qb + 1) * QB)
                bc = ps.tile([P, QB], F32, tag="ps", name=f"bc_{pr}_{qb}")
                nc.tensor.matmul(bc[:], sel_s[:], rd[:], start=True, stop=True)
                t1 = t1p.tile([P, QB], BF16, tag="t1", name=f"t1_{pr}_{qb}")
                nc.vector.tensor_tensor(t1[:], attn_raw[:, pr, qsl], bc[:],
                                        ALU.mult)
                nc.vector.tensor_scalar_add(t1[:], t1[:], ab1[:])
                nc.vector.tensor_tensor(attn_n[:, pr, qsl], t1[:],
                                        gateT[:, pr, qsl], ALU.mult)

            def push_tail(pr, qb):
                cell = {}
                def i1(pr=pr, qb=qb):
                    cell["rd"] = emit_tail1(pr, qb)
                def i2(pr=pr, qb=qb):
                    emit_tail2(pr, qb, cell.pop("rd"))
                push(f"tl1_{pr}_{qb}", i1)
                push(f"tl2_{pr}_{qb}", i2)

            def push_outproj(qb):
                for t in range(qb * NQB, (qb + 1) * NQB):
                    for n in range(E // 512):
                        def item(t=t, n=n):
                            pt = ps.tile([P, 512], F32, tag="ps",
                                         name=f"op_{t}_{n}")
                            for k in range(NC):
                                nc.tensor.matmul(
                                    pt[:],
                                    attn_n[:, k, t * P:(t + 1) * P],
                                    wo[:, k, n * 512:(n + 1) * 512],
                                    start=(k == 0), stop=(k == NC - 1))
                            ot = osp.tile([P, 512], F32, tag="os",
                                          name=f"os_{t}_{n}")
                            nc.vector.tensor_copy(ot[:], pt[:])
                            nc.sync.dma_start(
                                out_d[t * P:(t + 1) * P,
                                      n * 512:(n + 1) * 512], ot[:])
                        push(f"op_{t}_{n}", item)

            # ---------------- attention ----------------
            def evict_unit(pr, qb, acc):
                qsl = slice(qb * QB, (qb + 1) * QB)
                nc.vector.tensor_copy(attn_raw[0:D, pr, qsl], acc[0][0:D, :])
                tmpv = evp.tile([D, QB], BF16, tag="ev", name=f"ev_{pr}_{qb}")
                nc.vector.tensor_copy(tmpv[:], acc[1][0:D, :])
                nc.sync.dma_start(attn_raw[D:P, pr, qsl], tmpv[:])
                for h in range(2):
                    den = denp.tile([1, QB], F32R, tag="den",
                                    name=f"dn_{pr}_{qb}_{h}")
                    nc.vector.tensor_copy(den[:], acc[h][D:D + 1, :])
                    nc.sync.dma_start(rstage[:, 2 * pr + h, qb, :], den[:])

            def attn_unit(pr, qb, inline=None):
                qsl = slice(qb * QB, (qb + 1) * QB)
                acc = [accp.tile([D + 1, QB], F32, tag="acc",
                                 name=f"acc_{pr}_{qb}_{h}") for h in range(2)]
                ex = None
                for jc in range(NS):
                    sc = scp.tile([P, 2, QB], F32, tag="sc",
                                  name=f"sc_{pr}_{qb}_{jc}")
                    for h in range(2):
                        hb = h * D
                        nc.tensor.matmul(
                            sc[:, h, :],
                            kT[hb:hb + D, pr, jc * P:(jc + 1) * P],
                            qT[hb:hb + D, pr, qsl],
                            start=True, stop=True)
                    if jc % 2 == 0:
                        ex = exp_pool.tile([P, 2, 2, QB], BF16, tag="ex",
                                           name=f"ex_{pr}_{qb}_{jc}")
                    nc.scalar.activation(ex[:, jc % 2], sc[:], AF.Exp,
                                         scale=0.125)
                    for h in range(2):
                        nc.tensor.matmul(
                            acc[h][:],
                            v_aug[:, jc, 2 * pr + h, 0:D + 1],
                            ex[:, jc % 2, h, :],
                            start=(jc == 0), stop=(jc == NS - 1))
                    if inline is not None:
                        inline(jc)
                    else:
                        drip(1)
                evict_unit(pr, qb, acc)

            # ---------------- schedule ----------------
            # pre-phase: minimum to start (pr0, qb0)
            emit_kq_group(wkT, kT, bk, 0, 0, "k")
            emit_kq_group(wqT, qT, bq, 0, 0, "q")
            emit_v_chunk(0)
            emit_v_chunk(1)

            def pr0qb0_inline(jc):
                if jc <= 13:
                    emit_v_chunk(jc + 2)
                if jc == 1:
                    emit_kq_group(wkT, kT, bk, 0, 1, "k")
                elif jc == 5:
                    emit_kq_group(wkT, kT, bk, 0, 2, "k")
                elif jc == 9:
                    emit_kq_group(wkT, kT, bk, 0, 3, "k")

            def push_k(m, gs):
                for g in gs:
                    push_kq_group(wkT, kT, bk, m, g, "k")

            for pr in range(NC):
                for qb in range(NQB):
                    if pr == 0 and qb == 0:
                        attn_unit(0, 0, inline=pr0qb0_inline)
                    else:
                        if qb == 0:
                            ensure(f"k{pr}g{NQB - 1}")
                        emit_kq_group(wqT, qT, bq, pr, qb, "q")
                        attn_unit(pr, qb)

                    # pushes after unit (pr, qb)
                    if pr == 0 and qb == 0:
                        push_k(1, range(NQB))
                    elif pr == 0 and qb == 1:
                        push_k(2, [0, 1])
                    elif pr == 0 and qb == 2:
                        push_k(2, [2, 3])
                    elif pr == 0 and qb == 3:
                        push_h1(0)
                        push_gate(0, 0)
                        push_tail(0, 0)
                    elif pr == 1:
                        if qb < 3:
                            push_h1(qb + 1)
                            push_gate(0, qb + 1)
                            push_tail(0, qb + 1)
                            push_k(3, [qb])
                        else:
                            push_gate(1, 0)
                            push_tail(1, 0)
                            push_k(3, [3])
                    elif pr == 2:
                        if qb < 3:
                            push_gate(1, qb + 1)
                            push_tail(1, qb + 1)
                        else:
                            push_gate(2, 0)
                            push_tail(2, 0)
                    elif pr == 3:
                        if qb < 3:
                            push_gate(2, qb + 1)
                            push_tail(2, qb + 1)
                            push_gate(3, qb)
                            push_tail(3, qb)
                            push_outproj(qb)
                        else:
                            push_gate(3, 3)
                            push_tail(3, 3)
                            push_outproj(3)
            drain()

    nc.compile()
    return nc


_CACHE = {}


def _get_nc():
    if "nc" not in _CACHE:
        _CACHE["nc"] = build_nc()
    return _CACHE["nc"]


def _bf16_t(a):
    """transpose + cast to contiguous bf16"""
    return np.ascontiguousarray(
        np.asarray(a, np.float32).T).astype(ml_dtypes.bfloat16)


def kernel(query, Wq, bq, Wk, bk, Wv, bv, Wo, bo,
           Wm1, bm1, Wm2, bm2,
           dopamine, serotonin, norepinephrine, acetylcholine,
           attn_scale, attn_bias):
    B, S, E = 4, 2048, 1024
    CH = 512
    nc = _get_nc()

    query = np.asarray(query, np.float32)
    f32 = lambda a: np.ascontiguousarray(np.asarray(a, np.float32))
    scal_row = np.array([float(np.asarray(dopamine).reshape(-1)[0]),
                         float(np.asarray(serotonin).reshape(-1)[0]),
                         float(np.asarray(norepinephrine).reshape(-1)[0]),
                         float(np.asarray(acetylcholine).reshape(-1)[0]),
                         float(np.asarray(attn_scale).reshape(-1)[0]),
                         float(np.asarray(attn_bias).reshape(-1)[0]),
                         0.0, 0.0], np.float32)
    scal = np.tile(scal_row[None, :], (128, 1))
    D_ = 64
    sel = np.zeros((2, 128), np.float32)
    sel[0, 0:D_] = 1.0
    sel[1, D_:2 * D_] = 1.0

    wm1T = _bf16_t(Wm1)
    Wo_np = np.asarray(Wo, np.float32)
    in_maps = []
    for core in range(8):
        b, g = core // 2, core % 2
        cg = slice(g * CH, (g + 1) * CH)
        in_maps.append({
            "xT": _bf16_t(query[b]),
            "wqT": _bf16_t(np.asarray(Wq, np.float32)[cg]),
            "wkT": _bf16_t(np.asarray(Wk, np.float32)[cg]),
            "wvT": _bf16_t(np.asarray(Wv, np.float32)[cg]),
            "wm1T": wm1T,
            "wm2T": _bf16_t(np.asarray(Wm2, np.float32)[cg]),
            "wo": _bf16_t(Wo_np[:, cg]),
            "bq": f32(np.asarray(bq, np.float32)[cg]),
            "bk": f32(np.asarray(bk, np.float32)[cg]),
            "bvr": np.ascontiguousarray(
                np.tile(np.asarray(bv, np.float32)[cg][None, :], (128, 1))),
            "bm1": f32(bm1),
            "bm2": f32(np.asarray(bm2, np.float32)[cg]),
            "scal": scal,
            "sel": sel,
        })

    res = run_bass_kernel_spmd(nc, in_maps, core_ids=list(range(8)))
    _CACHE["last_results"] = res

    bo_np = np.asarray(bo, np.float32)
    out = np.empty((B, S, E), np.float32)
    for b in range(B):
        out[b] = res.results[2 * b]["out"] + res.results[2 * b + 1]["out"] + bo_np
    return out
